# revision 24
# baseline (speedup 1.0000x reference)
"""Trainium2 Bass kernel for nn_BiLSTMNet (2-layer BiLSTM + path-gather + MLP + softmax).

Sharding: data-parallel over batch B=128 across 8 cores (16 samples/core).
All weights replicated. Each core computes its batch shard end-to-end; host
concatenates the per-core [BL*P, C] outputs.

v2 design (vs v1):
  - Projection matmuls write gate windows (WIN=8 steps) DIRECTLY into PSUM;
    the recurrence whh matmuls accumulate on top (start=False). No identity
    preload matmul, no PSUM->SBUF prew staging.
  - tanh via sigmoid: g-gate rows of all weights scaled x2 on host, h stored
    as h/2 with all h-consumers (whh, wih_l1, w1) scaled x2. One Sigmoid over
    all 128 gate cols + one Sigmoid(2c) per step on ACT; 3 fused
    scalar_tensor_tensor ops on DVE + 1 tensor_mul on GpSimd per step.
  - Embedding gather done host-side (xrow input, row-major [NT, 256] bf16 with
    a ones column at 200 that carries the bias via an extra contraction row).
  - All transposes on the DMA crossbar (dma_start_transpose): x window loads
    transpose straight out of DRAM; h1 export transposes SBUF->SBUF.
  - MLP unchanged in spirit: row-gather from h1r, PE transposes (PSUM is free
    after the recurrence), two matmuls, softmax.
"""

import numpy as np
import ml_dtypes

import concourse.bass as bass
import concourse.mybir as mybir
import concourse.tile as tile
from concourse import bacc
from concourse._compat import with_exitstack
from concourse.masks import make_identity

F32 = mybir.dt.float32
BF16 = mybir.dt.bfloat16
I32 = mybir.dt.int32
AF = mybir.ActivationFunctionType
ALU = mybir.AluOpType
BF16NP = ml_dtypes.bfloat16

# problem constants
V, E, H, T_FULL, B, PP, MLPD, C = 30000, 200, 200, 512, 128, 256, 200, 4
NCORES = 8
BL = B // NCORES          # 16 samples per core
GP = 8                    # padded gate groups (i0,i1,f0,f1,o0,o1,g0,g1)
KC = (128, 72)            # H contraction chunks
WIN = 8                   # steps per window (window = 2 PSUM banks per dir)
WTOK = WIN * BL           # 128 tokens per window
WB = 4                    # windows per input-load batch
RS0 = 32                  # layer-0 h ring length (steps)
DIRS = ("f", "b")


# ---------------------------------------------------------------- host packing

def _pack_gate_rows(w):
    """[800, ...] pytorch gate order (i,f,g,o) -> [1024, ...] order (i,f,g,o),
    each gate split into (128, 72+56pad) groups."""
    i, f, g, o = w[0:200], w[200:400], w[400:600], w[600:800]
    parts = []
    for gate in (i, f, g, o):
        parts.append(gate[0:128])
        pad = np.zeros((56,) + gate.shape[1:], np.float32)
        parts.append(np.concatenate([gate[128:200], pad], 0))
    return np.concatenate(parts, 0)


def prep_weights(inp):
    """Host-side packing of all weights. Returns dict of np arrays (shared by all cores)."""
    w = {}
    for layer in (0, 1):
        for d in DIRS:
            nm = f"l{layer}_{d}"
            wih = np.asarray(inp["wih_" + nm], np.float32).copy()
            whh = np.asarray(inp["whh_" + nm], np.float32).copy()
            bias = (np.asarray(inp["bih_" + nm], np.float32)
                    + np.asarray(inp["bhh_" + nm], np.float32)).copy()
            # h is stored halved -> double every consumer of h
            whh *= 2.0
            if layer == 1:
                wih *= 2.0
            # tanh-via-sigmoid: double g-gate rows (pytorch order i,f,g,o)
            wih[400:600] *= 2.0
            whh[400:600] *= 2.0
            bias[400:600] *= 2.0
            wihp = _pack_gate_rows(wih)               # [1024, Din]
            whhp = _pack_gate_rows(whh)               # [1024, 200]
            bp = _pack_gate_rows(bias[:, None])[:, 0]  # [1024]
            wihT = np.ascontiguousarray(wihp.T)       # [Din, 1024]
            whhT = np.ascontiguousarray(whhp.T)       # [200, 1024]
            # K-chunks; bias row appended to chunk 1 (contraction row = const 1)
            nkin = 2 if layer == 0 else 4
            for ci in range(nkin):
                lo = 200 * (ci // 2) + 128 * (ci % 2)
                hi = lo + (128 if ci % 2 == 0 else 72)
                chunk = wihT[lo:hi]
                if ci == 1:
                    chunk = np.concatenate([chunk, bp[None, :]], 0)  # [73, 1024]
                w[f"wih_{nm}_k{ci}"] = np.ascontiguousarray(chunk).astype(BF16NP)
            w[f"whh_{nm}_k0"] = np.ascontiguousarray(whhT[0:128]).astype(BF16NP)
            w[f"whh_{nm}_k1"] = np.ascontiguousarray(whhT[128:200]).astype(BF16NP)
    # MLP: w1 consumes stored h1 (halved) -> x2
    w1T = (np.asarray(inp["w1"], np.float32) * 2.0).T     # [800, 200]
    for j in range(8):
        lo = 200 * (j // 2) + 128 * (j % 2)
        hi = lo + (128 if j % 2 == 0 else 72)
        w[f"w1_c{j}"] = np.ascontiguousarray(w1T[lo:hi]).astype(BF16NP)
    b1 = np.asarray(inp["b1"], np.float32)
    b1p = np.zeros((128, 2), np.float32)
    b1p[:, 0] = b1[0:128]
    b1p[0:72, 1] = b1[128:200]
    w["b1"] = b1p
    w2T = np.asarray(inp["w2"], np.float32).T             # [200, 4]
    w["w2_k0"] = np.ascontiguousarray(w2T[0:128]).astype(BF16NP)
    w["w2_k1"] = np.ascontiguousarray(w2T[128:200]).astype(BF16NP)
    w["b2"] = np.tile(np.asarray(inp["b2"], np.float32)[None, :], (128, 1))  # [128, 4]
    return w


def prep_core_inputs(inp, wshared, core, T, xfull):
    """Per-core input map: shared weights + this core's x rows / path indices."""
    b0 = core * BL
    NT = T * BL
    m = dict(wshared)
    # x rows, t-major: row t*BL+b = x[t, b0+b]; col 200 = 1.0 (bias row source)
    xc = xfull[:T, b0:b0 + BL, :].reshape(NT, E)
    xrow = np.zeros((NT, 256), BF16NP)
    xrow[:, 0:E] = xc.astype(BF16NP)
    xrow[:, E] = BF16NP(1.0)
    m["xrow"] = xrow
    # path gather indices into h1r rows (t-major slots); invalid -> NT (zero row)
    paths = np.asarray(inp["paths"], np.int64)[b0:b0 + BL]   # [BL, P, 2]
    bcol = np.arange(BL, dtype=np.int64)[:, None, None]
    idx = np.where(paths >= 0, BL * paths + bcol, NT)
    nel = BL * PP
    ptile = nel // 128
    for k in range(2):
        fk = idx[:, :, k].reshape(nel).astype(np.int32)
        m[f"path_idx_k{k}"] = np.ascontiguousarray(fk.reshape(ptile, 128).T)  # [128, ptile]
    return m


# ---------------------------------------------------------------- device kernel

@with_exitstack
def bilstm_kernel(ctx, tc, io, T, dump=False):
    nc = tc.nc
    NT = T * BL
    NW = T // WIN                      # windows per direction
    NB = NW // WB                      # input-load batches per direction
    nel = BL * PP

    const = ctx.enter_context(tc.tile_pool(name="const", bufs=1))
    ident_bf = const.tile([128, 128], BF16)
    make_identity(nc, ident_bf[:])

    # ---- load weights to SBUF (layer-0 weights first so the pipeline starts
    # early; everything else is issued after the first input batches)
    sb = {}
    KIH = {0: (128, 73), 1: (128, 73, 128, 72)}

    def load_layer_weights(layer):
        for d in DIRS:
            nm = f"l{layer}_{d}"
            for ci, kc in enumerate(KIH[layer]):
                t = const.tile([kc, 1024], BF16, tag=f"wih{nm}{ci}", name=f"wih{nm}{ci}")
                nc.sync.dma_start(t[:], io[f"wih_{nm}_k{ci}"][:])
                sb[f"wih_{nm}_k{ci}"] = t
            for ci in range(2):
                t = const.tile([KC[ci], 1024], BF16, tag=f"whh{nm}{ci}", name=f"whh{nm}{ci}")
                nc.sync.dma_start(t[:], io[f"whh_{nm}_k{ci}"][:])
                sb[f"whh_{nm}_k{ci}"] = t

    load_layer_weights(0)
    ones_sb = const.tile([1, 256], BF16, tag="ones", name="ones")
    nc.gpsimd.memset(ones_sb[:], 1.0)
    zrow = const.tile([128, 512], BF16, tag="zrow", name="zrow")
    nc.gpsimd.memset(zrow[:], 0.0)
    ptile = nel // 128
    pidx = {}

    def load_rest():
        load_layer_weights(1)
        for j in range(8):
            kc = 128 if j % 2 == 0 else 72
            t = const.tile([kc, MLPD], BF16, tag=f"w1c{j}", name=f"w1c{j}")
            nc.sync.dma_start(t[:], io[f"w1_c{j}"][:])
            sb[f"w1_c{j}"] = t
        for nm, shp, dt in (("b1", [128, 2], F32), ("w2_k0", [128, 4], BF16),
                            ("w2_k1", [72, 4], BF16), ("b2", [128, 4], F32)):
            t = const.tile(shp, dt, tag=nm, name=nm + "_s")
            nc.sync.dma_start(t[:], io[nm][:])
            sb[nm] = t
        for k in range(2):
            pidx[k] = const.tile([128, ptile], I32, tag=f"pidx{k}", name=f"pidx{k}")
            nc.sync.dma_start(pidx[k][:], io[f"path_idx_k{k}"][:])

    # ---- persistent SBUF state
    big = ctx.enter_context(tc.tile_pool(name="big", bufs=1))
    ring0 = {d: big.tile([128, 2 * RS0 * BL], BF16, tag=f"ring0{d}", name=f"ring0{d}")
             for d in DIRS}
    ring1 = {d: big.tile([128, 2 * NT], BF16, tag=f"ring1{d}", name=f"ring1{d}")
             for d in DIRS}
    cst = {d: big.tile([128, 32], F32, tag=f"c{d}", name=f"c{d}") for d in DIRS}

    # ---- DRAM scratch
    knd = "ExternalOutput" if dump else "Internal"
    h0_dram = {d: nc.dram_tensor(f"h0_sc_{d}", [2, 128, NT], BF16, kind=knd).ap()
               for d in DIRS}
    h1r = nc.dram_tensor("h1r", [NT + 1, 512], BF16, kind=knd).ap()

    # ---------------- recurrence phase (scoped PSUM pools: 8 banks for windows)
    with tc.tile_pool(name="pswf", bufs=2, space="PSUM") as pswf, \
         tc.tile_pool(name="pswb", bufs=2, space="PSUM") as pswb, \
         tc.tile_pool(name="inw", bufs=3) as inw_pool, \
         tc.tile_pool(name="gates", bufs=8) as gpool, \
         tc.tile_pool(name="h1stg", bufs=3) as stg_pool:
        psw = {"f": pswf, "b": pswb}

        def load_batch(layer, d, j):
            """DMA the input rows for load-batch j (WB windows) of direction d."""
            nch = 2 if layer == 0 else 4
            tl = inw_pool.tile([128, nch * WB * WTOK], BF16, tag=f"inw{layer}{d}",
                               name=f"inw{layer}{d}")
            view = tl[:, :].rearrange("p (c n) -> p c n", c=nch)
            r0 = 512 * j if d == "f" else NT - 512 * (j + 1)
            if layer == 0:
                nc.sync.dma_start_transpose(view[:, 0, :], io["xrow"][r0:r0 + 512, 0:128])
                nc.sync.dma_start_transpose(view[:, 1, :], io["xrow"][r0:r0 + 512, 128:256])
            else:
                for di, dd in enumerate(DIRS):
                    nc.sync.dma_start(
                        view[:, 2 * di:2 * di + 2, :],
                        h0_dram[dd][:, :, r0:r0 + 512].rearrange("c p n -> p c n"))
            return tl

        def proj_thunks(layer, d, w, inw, PW):
            """Per-gate-group emit closures for window w's projection matmuls."""
            nm = f"l{layer}_{d}"
            nch = 2 if layer == 0 else 4
            view = inw[:, :].rearrange("p (c n) -> p c n", c=nch)
            if d == "f":
                blk = w % WB
            else:
                blk = WB - 1 - (w % WB)

            def mk(g, cis):
                def emit():
                    for ci in cis:
                        kc = KIH[layer][ci]
                        nc.tensor.matmul(
                            PW[:, 128 * g:128 * (g + 1)],
                            sb[f"wih_{nm}_k{ci}"][:, 128 * g:128 * (g + 1)],
                            view[0:kc, ci, WTOK * blk:WTOK * (blk + 1)],
                            start=(ci == 0 and g % 4 == 0), stop=False,
                            skip_group_check=True)
                return emit

            if layer == 0:
                return [mk(g, (0, 1)) for g in range(GP)]
            # L1: split each group's 4 matmuls into 2 emit-granules
            out = []
            for g in range(GP):
                out.append(mk(g, (0, 1)))
                out.append(mk(g, (2, 3)))
            return out

        def rec_pre(layer, d, sd, ring, RS):
            """Matmuls + gate sigmoid for direction d's step sd."""
            w, tau = sd // WIN, sd % WIN
            t = sd if d == "f" else T - 1 - sd
            PW = PWs[(d, w)]
            nm = f"l{layer}_{d}"
            # window column = position of t within the window in ascending-token
            # order (b consumes its window time-reversed)
            col = tau if d == "f" else WIN - 1 - tau
            tprev = (t - 1 if d == "f" else t + 1) % RS
            if sd != 0:
                # chunk-0 matmuls first: they only need ring chunk 0, which the
                # split h-write lands one DVE op earlier
                for ci in range(2):
                    kc = KC[ci]
                    rhs = ring[0:kc, RS * BL * ci + BL * tprev:
                               RS * BL * ci + BL * (tprev + 1)]
                    for g in range(GP):
                        nc.tensor.matmul(
                            PW[:, 128 * g + 16 * col:128 * g + 16 * (col + 1)],
                            sb[f"whh_{nm}_k{ci}"][:, 128 * g:128 * (g + 1)],
                            rhs, start=False,
                            stop=(tau == WIN - 1 and ci == 1 and g % 4 == 3),
                            skip_group_check=True)
            sg = gpool.tile([128, 128], F32, tag=f"sg{d}", name=f"sg{d}")
            view = PW[:, :].rearrange("p (g n) -> p g n", g=GP)
            nc.scalar.activation(sg[:, :].rearrange("p (g n) -> p g n", g=GP),
                                 view[:, :, 16 * col:16 * (col + 1)], AF.Sigmoid)
            return {"sg": sg, "t": t}

        def rec_mid(d, st):
            """Cell-state update + tanh."""
            sg = st["sg"]
            c1 = gpool.tile([128, 32], F32, tag=f"c1{d}", name=f"c1{d}")
            t1 = gpool.tile([128, 32], F32, tag=f"t1{d}", name=f"t1{d}")
            sc = gpool.tile([128, 32], F32, tag=f"sc{d}", name=f"sc{d}")
            # c = sig(f)*c + tanh(g)*sig(i);  tanh(g) = 2*(sig(2g)-0.5)
            nc.vector.tensor_mul(c1[:], sg[:, 32:64], cst[d][:])
            nc.vector.scalar_tensor_tensor(t1[:], sg[:, 64:96], 0.5, sg[:, 0:32],
                                           ALU.subtract, ALU.mult)
            nc.vector.scalar_tensor_tensor(cst[d][:], t1[:], 2.0, c1[:],
                                           ALU.mult, ALU.add)
            nc.scalar.activation(sc[:], cst[d][:], AF.Tanh)
            st["sc"] = sc

        def rec_post(d, st, ring, RS):
            """h/2 = sig(o)*tanh(c)*0.5, written per h-chunk so the next step's
            chunk-0 matmuls can start while chunk 1 is still being written."""
            sg, sc = st["sg"], st["sc"]
            rp = st["t"] % RS
            for c2 in range(2):
                nc.vector.scalar_tensor_tensor(
                    ring[:, RS * BL * c2 + BL * rp:RS * BL * c2 + BL * (rp + 1)],
                    sc[:, 16 * c2:16 * (c2 + 1)], 0.5,
                    sg[:, 96 + 16 * c2:96 + 16 * (c2 + 1)],
                    ALU.mult, ALU.mult)

        def export_h0(d, w):
            """Export layer-0 h (2 windows = 16 steps) to h0_dram."""
            t0 = WIN * (w - 1) if d == "f" else T - WIN * (w + 1)
            rp0 = t0 % RS0
            view = ring0[d][:, :].rearrange("p (c n) -> p c n", c=2)
            sl = slice(BL * t0, BL * (t0 + 2 * WIN))
            rsl = slice(BL * rp0, BL * (rp0 + 2 * WIN))
            if d == "b":
                nc.sync.dma_start(
                    h0_dram[d][:, :, sl].rearrange("c p n -> p c n"), view[:, :, rsl])
            else:
                # keep row 72 of chunk 1 disjoint: it carries the L1 bias ones
                nc.sync.dma_start(
                    h0_dram[d][0, :, sl].rearrange("p n -> p n"), view[:, 0, rsl])
                nc.sync.dma_start(h0_dram[d][1, 0:72, sl], view[0:72, 1, rsl])
                nc.sync.dma_start(h0_dram[d][1, 72:73, sl], ones_sb[0:1, 0:BL * 2 * WIN])

        def export_h1(bi):
            """Transpose one 128-slot block of both layer-1 rings into h1r rows."""
            stage = stg_pool.tile([128, 512], BF16, tag="h1s", name="h1s")
            for di, d in enumerate(DIRS):
                for c2 in range(2):
                    src = ring1[d][:, NT * c2 + 128 * bi:NT * c2 + 128 * (bi + 1)]
                    nc.sync.dma_start_transpose(
                        stage[:, 256 * di + 128 * c2:256 * di + 128 * (c2 + 1)], src)
            nc.sync.dma_start(h1r[128 * bi:128 * (bi + 1), :], stage[:, :])

        # The b chain runs LAG steps behind f: b's matmul bursts then never
        # head-of-line-block f's next burst in the in-order PE queue, so the
        # pair period is set by one chain's latency, not both bursts.
        LAG = 6
        for layer in (0, 1):
            RS = RS0 if layer == 0 else T
            ring = ring0 if layer == 0 else ring1
            for d in DIRS:
                nc.vector.memset(cst[d][:], 0.0)
            batches = {}
            for d in DIRS:
                batches[(d, 0)] = load_batch(layer, d, 0)
            if layer == 0:
                load_rest()

            PWs = {}
            pending = {"f": [], "b": []}

            def mkPW(d, w):
                PW = psw[d].tile([128, GP * WTOK], F32, tag=f"win{d}", name=f"win{d}")
                PWs[(d, w)] = PW
                return proj_thunks(layer, d, w, batches[(d, w // WB)], PW)

            for d in DIRS:
                for th in mkPW(d, 0):
                    th()

            def dir_tail(d, sd):
                """Window bookkeeping after direction d's step sd."""
                w, tau = sd // WIN, sd % WIN
                if tau == 0 and w + 1 < NW:
                    j = (w + 2) // WB
                    if (w + 2) % WB == 0 and j < NB:
                        batches[(d, j)] = load_batch(layer, d, j)
                    pending[d] = mkPW(d, w + 1)
                npop = 1 if layer == 0 else 2
                for _ in range(npop):
                    if pending[d]:
                        pending[d].pop(0)()
                if tau == WIN - 1:
                    PWs.pop((d, w))
                    if layer == 0 and w % 2 == 1:
                        export_h0(d, w)

            # phase-interleaved emission: both chains' sigmoids are queued on
            # ACT before either tanh, and both c-updates on DVE before either
            # h-write, so a ready op never sits behind the other chain's stall
            for s in range(T + LAG):
                stf = rec_pre(layer, "f", s, ring["f"], RS) if s < T else None
                stb = rec_pre(layer, "b", s - LAG, ring["b"], RS) if s >= LAG else None
                if stf:
                    rec_mid("f", stf)
                if stb:
                    rec_mid("b", stb)
                if stf:
                    rec_post("f", stf, ring["f"], RS)
                    dir_tail("f", s)
                if stb:
                    rec_post("b", stb, ring["b"], RS)
                    dir_tail("b", s - LAG)
                if layer == 1:
                    for bi in range(NW):
                        if s == max(WIN * bi + WIN - 1, T + LAG - 1 - WIN * bi):
                            export_h1(bi)

    # ---------------- MLP + softmax (PSUM pools reopened after rec pools close)
    with tc.tile_pool(name="psm1", bufs=2, space="PSUM") as psm1, \
         tc.tile_pool(name="psm2", bufs=2, space="PSUM") as psm2, \
         tc.tile_pool(name="pst", bufs=4, space="PSUM") as pst, \
         tc.tile_pool(name="mlp", bufs=2) as mpool, \
         tc.tile_pool(name="gath", bufs=4) as gath, \
         tc.tile_pool(name="osm", bufs=4) as opool:
        nc.sync.dma_start(h1r[NT:NT + 1, :], zrow[0:1, :])
        nblk = nel // 128
        for e in range(nblk):
            mlpT = mpool.tile([128, 8 * 128], BF16, tag="mlpT", name="mlpT")
            for k in range(2):
                gt = gath.tile([128, 512], BF16, tag="g", name="gt")
                nc.gpsimd.indirect_dma_start(
                    out=gt[:], out_offset=None, in_=h1r[:],
                    in_offset=bass.IndirectOffsetOnAxis(ap=pidx[k][:, e:e + 1], axis=0),
                    bounds_check=NT, oob_is_err=False)
                for f4 in range(4):
                    pt = pst.tile([128, 128], BF16, tag="tp", name="tpb")
                    nc.tensor.transpose(pt[:], gt[:, 128 * f4:128 * (f4 + 1)], ident_bf[:])
                    nc.vector.tensor_copy(mlpT[:, 128 * (4 * k + f4):128 * (4 * k + f4 + 1)],
                                          pt[:])
            hidT = mpool.tile([128, 2 * 128], BF16, tag="hidT", name="hidT")
            for m in range(2):
                pm = KC[m]
                ps1 = psm1.tile([128, 128], F32, tag="mm1", name="mm1ps")
                for j in range(8):
                    kc = 128 if j % 2 == 0 else 72
                    nc.tensor.matmul(ps1[:pm, :], sb[f"w1_c{j}"][:kc, 128 * m:128 * m + pm],
                                     mlpT[0:kc, 128 * j:128 * (j + 1)],
                                     start=(j == 0), stop=(j == 7))
                nc.scalar.activation(hidT[:pm, 128 * m:128 * (m + 1)], ps1[:pm, :],
                                     AF.Tanh, bias=sb["b1"][:pm, m:m + 1])
            ps2 = psm2.tile([128, 4], F32, tag="mm2", name="mm2ps")
            for ci in range(2):
                kc = KC[ci]
                nc.tensor.matmul(ps2[:], hidT[:kc, 128 * ci:128 * ci + 128],
                                 sb[f"w2_k{ci}"][:], start=(ci == 0), stop=(ci == 1))
            lg = opool.tile([128, 4], F32, tag="lg", name="lg")
            ex = opool.tile([128, 4], F32, tag="ex", name="ex")
            sm = opool.tile([128, 1], F32, tag="sm", name="sm")
            rc = opool.tile([128, 1], F32, tag="rc", name="rc")
            ot = opool.tile([128, 4], F32, tag="ot", name="ot")
            nc.vector.tensor_add(lg[:], ps2[:], sb["b2"][:])
            nc.scalar.activation(ex[:], lg[:], AF.Exp)
            nc.vector.tensor_reduce(sm[:], ex[:], axis=mybir.AxisListType.X,
                                    op=mybir.AluOpType.add)
            nc.vector.reciprocal(rc[:], sm[:])
            nc.vector.tensor_scalar_mul(ot[:], ex[:], rc[:])
            nc.sync.dma_start(io["out"][128 * e:128 * (e + 1), :], ot[:])


# ---------------------------------------------------------------- build + run

def build(T=T_FULL, do_compile=True, dump=False):
    nc = bacc.Bacc("TRN2", target_bir_lowering=False, debug=False)
    NT = T * BL
    nel = BL * PP
    io = {}

    def din(name, shape, dtype):
        io[name] = nc.dram_tensor(name, list(shape), dtype, kind="ExternalInput").ap()

    din("xrow", (NT, 256), BF16)
    for k in range(2):
        din(f"path_idx_k{k}", (128, nel // 128), I32)
    KIH = {0: (128, 73), 1: (128, 73, 128, 72)}
    for layer in (0, 1):
        for d in DIRS:
            nm = f"l{layer}_{d}"
            for ci, kc in enumerate(KIH[layer]):
                din(f"wih_{nm}_k{ci}", (kc, 1024), BF16)
            for ci in range(2):
                din(f"whh_{nm}_k{ci}", (KC[ci], 1024), BF16)
    for j in range(8):
        din(f"w1_c{j}", (128 if j % 2 == 0 else 72, MLPD), BF16)
    din("b1", (128, 2), F32)
    din("w2_k0", (128, 4), BF16)
    din("w2_k1", (72, 4), BF16)
    din("b2", (128, 4), F32)
    io["out"] = nc.dram_tensor("out", [nel, C], F32, kind="ExternalOutput").ap()

    with tile.TileContext(nc) as tc:
        bilstm_kernel(tc, io, T, dump=dump)
    if do_compile:
        nc.compile()
    return nc


_CACHED = {}


def kernel(**inputs):
    tokens = np.asarray(inputs["tokens"], np.int64)
    T = tokens.shape[0]
    if T not in _CACHED:
        _CACHED[T] = build(T)
    nc = _CACHED[T]
    wshared = prep_weights(inputs)
    xfull = np.asarray(inputs["emb"], np.float32)[tokens]   # [T, B, E]
    in_maps = [prep_core_inputs(inputs, wshared, core, T, xfull) for core in range(NCORES)]
    from concourse.bass_utils import run_bass_kernel_spmd
    res = run_bass_kernel_spmd(nc, in_maps, core_ids=list(range(NCORES)))
    return np.concatenate([res.results[i]["out"] for i in range(NCORES)], 0)
